# revision 10
# baseline (speedup 1.0000x reference)
"""Trainium2 Bass kernel for nn_Attention_85074712199827.

Computes, for hidden [1,32,1024], encoder_outputs [32,2048,1024],
W_attn [1024,2048], b_attn [1024], v [1024]:

    h_proj  = hidden[0] @ W_attn[:, :1024].T
    e_proj  = encoder_outputs @ W_attn[:, 1024:].T
    energy  = tanh(e_proj + h_proj[:, None, :] + b_attn)
    att     = energy @ v
    out     = softmax(att, axis=1)          # [32, 2048] float32

Distribution: data-parallel over the batch across 8 NeuronCores (4
batch rows per core); the tiny parameters are replicated (pre-laid-out
and pre-cast to bf16 on the host). Each core runs an independent
Bass/Tile program; results are concatenated on the host.

Per-core schedule: enc streams HBM->SBUF fp32 on the sync HWDGE queue
(loads only -- nothing else shares that ring), is cast to bf16 (DVE),
and xbar-transposed to [h, s] layout on the scalar HWDGE queue.  The
e_proj matmuls run in 1024-wide units with the stationary weight block
reused across the two 512-wide PSUM halves, halving LDWEIGHTS
pressure (each LDWEIGHTS steals ~50ns from the concurrent matmul).
The v-dot is NOT done with 1-row PE matmuls: the tanh output is scaled
by v on the scalar engine, the 8 o-chunks are summed on DVE, and a
single ones-vector matmul reduces the 128 partitions.  Softmax runs
per batch row as soon as that row's attention logits are complete,
overlapped with the next row's matmuls.

Self-contained: only environment packages (concourse, numpy, ml_dtypes)
are imported; all shapes/sharding are hardcoded for this problem.
"""

from contextlib import ExitStack

import ml_dtypes
import numpy as np

import concourse.bass as bass
import concourse.tile as tile
from concourse import bacc, mybir

F32 = mybir.dt.float32
BF16 = mybir.dt.bfloat16
AF = mybir.ActivationFunctionType
ADD = mybir.AluOpType.add
P = 128


def build_nc(b_loc=4, s=2048, h=1024, n_cores=8, sb=1024,
             warmup_mm=34, startup_keep=8, keepalive_mm=0,
             first_sb=512, ld_chunk=512):
    n_hc = h // P           # contraction chunks
    n_ot = h // P           # output (o) tiles

    nc = bacc.Bacc("TRN2", target_bir_lowering=False, debug=False,
                   num_devices=n_cores)

    wt = nc.dram_tensor("wt", [2 * h, h], BF16, kind="ExternalInput").ap()
    hiddenT = nc.dram_tensor("hiddenT", [h, b_loc], BF16, kind="ExternalInput").ap()
    b_attn = nc.dram_tensor("b_attn", [h], F32, kind="ExternalInput").ap()
    v = nc.dram_tensor("v", [h], F32, kind="ExternalInput").ap()
    ones = nc.dram_tensor("ones", [P, 1], BF16, kind="ExternalInput").ap()
    zeros = nc.dram_tensor("zeros", [P, 512], BF16, kind="ExternalInput").ap()
    enc = nc.dram_tensor("enc", [b_loc, s, h], F32, kind="ExternalInput").ap()
    out = nc.dram_tensor("out", [b_loc, s], F32, kind="ExternalOutput").ap()

    with tile.TileContext(nc) as tc, ExitStack() as ctx:
        const = ctx.enter_context(tc.tile_pool(name="const", bufs=1))
        psmall = ctx.enter_context(tc.tile_pool(name="psmall", bufs=1, space="PSUM"))

        # ---- PE warmup: dependency-free matmuls to lift the HAM clock
        # gate to 8/8 while the first enc block is still in flight ----
        wz = const.tile([P, 512], BF16)
        nc.scalar.dma_start(wz[:], zeros)
        for i in range(warmup_mm):
            pw = psmall.tile([P, 512], F32, name="pw", tag="ps")
            nc.tensor.matmul(pw[:], wz[:, :P], wz[:], start=True, stop=True)

        def keepalive(n):
            for _ in range(n):
                pw = psmall.tile([P, 512], F32, name="pw", tag="ps")
                nc.tensor.matmul(pw[:], wz[:, :P], wz[:], start=True, stop=True)

        # ---- small constants (scalar queue; off the critical path) ----
        hT_bf = const.tile([P, n_hc, b_loc], BF16)
        nc.scalar.dma_start(hT_bf[:], hiddenT.rearrange("(hc p) b -> p hc b", p=P))

        baT = const.tile([P, n_ot], F32)
        nc.scalar.dma_start(baT[:], b_attn.rearrange("(oc p) -> p oc", p=P))

        vt_f = const.tile([P, n_ot], F32)
        nc.scalar.dma_start(vt_f[:], v.rearrange("(oc p) -> p oc", p=P))

        ones_bf = const.tile([P, 1], BF16)
        nc.scalar.dma_start(ones_bf[:], ones)

        # ---- weights: W_attn.T arrives [2h, h] bf16; We half first so
        # e_proj unblocks while Wh still streams ----
        wt_bf = const.tile([P, 2 * n_hc, h], BF16)
        wt_r = wt.rearrange("(jc p) o -> p jc o", p=P)
        q = n_hc // 2

        def emit_w(c):
            nc.scalar.dma_start(
                wt_bf[:, c * q:(c + 1) * q, :],
                wt_r[:, c * q:(c + 1) * q, :])

        emit_w(2)
        emit_w(3)

        def emit_hproj():
            hb = const.tile([P, n_ot, b_loc], F32, name="hb")
            for ot in range(n_ot):
                ph = psmall.tile([P, b_loc], F32, name="ph", tag="ps")
                for hc in range(n_hc):
                    nc.tensor.matmul(
                        ph[:], wt_bf[:, hc, ot * P:(ot + 1) * P], hT_bf[:, hc, :],
                        start=(hc == 0), stop=(hc == n_hc - 1))
                nc.vector.tensor_tensor(
                    hb[:, ot, :], ph[:],
                    baT[:, ot, None].to_broadcast((P, b_loc)),
                    mybir.AluOpType.add)
            return hb

        # ---- main pipeline pools ----
        inp = ctx.enter_context(tc.tile_pool(name="inp", bufs=2))
        bfp = ctx.enter_context(tc.tile_pool(name="bfp", bufs=4))
        encT_p = ctx.enter_context(tc.tile_pool(name="encT", bufs=2))
        en_p = ctx.enter_context(tc.tile_pool(name="energy", bufs=3))
        tmp_p = ctx.enter_context(tc.tile_pool(name="vtmp", bufs=3))
        acc_p = ctx.enter_context(tc.tile_pool(name="acc", bufs=3))
        row_p = ctx.enter_context(tc.tile_pool(name="rowbuf", bufs=1))
        pe_p = ctx.enter_context(tc.tile_pool(name="psum_e", bufs=2, space="PSUM"))
        pa_p = ctx.enter_context(tc.tile_pool(name="psum_att", bufs=2, space="PSUM"))

        # per-row logits buffers (DVE/ACT accesses must start at partition 0)
        att_rows = [const.tile([1, s], F32, name=f"attrow{b}")
                    for b in range(b_loc)]

        # units: (b, s0, size) — b-major so softmax(b) pipelines.
        # The first units of b=0 are small for a fast pipeline rampup.
        units = []
        for b in range(b_loc):
            if b == 0 and first_sb < sb:
                for s0 in range(0, sb, first_sb):
                    units.append((b, s0, first_sb))
                for s0 in range(sb, s, sb):
                    units.append((b, s0, sb))
            else:
                for s0 in range(0, s, sb):
                    units.append((b, s0, sb))

        def phase1(unit):
            # HBM -> SBUF fp32 loads on the sync HWDGE queue (SWDGE issue
            # on gpsimd measured 10-40us/load), then DVE casts to bf16.
            # The very first unit loads in 128-row chunks so its first
            # transpose can start ~3us in instead of ~8us.
            b, s0, sz = unit
            chunk = P if (b == 0 and s0 == 0) else ld_chunk
            its = []
            for c0 in range(0, sz, chunk):
                csz = min(chunk, sz - c0)
                it = inp.tile([P, csz // P, h], F32, name="it")
                nc.sync.dma_start(
                    it[:], enc[b, s0 + c0:s0 + c0 + csz, :].rearrange(
                        "(si p) h -> p si h", p=P))
                its.append(it)
            bts = []
            for it in its:
                bt = bfp.tile([P, it.shape[1], h], BF16, name="bt")
                nc.vector.tensor_copy(out=bt[:], in_=it[:])
                bts.append(bt)
            return bts

        def phase2(unit, bts):
            # SBUF xbar transpose [s,h] bf16 -> [h,s] on the scalar HWDGE
            # queue so the sync ring only ever carries the enc loads.
            b, s0, sz = unit
            eT = encT_p.tile([P, n_hc, sz], BF16, name="eT")
            col = 0
            for bt in bts:
                for si in range(bt.shape[1]):
                    nc.scalar.dma_start_transpose(
                        eT[:, :, col:col + P], bt[:, si, :])
                    col += P
            return eT

        def phase3_mm(unit, eT, hb):
            # PSUM matmul output must fit one 2KB bank -> 512-wide chunks,
            # but the stationary weight block is reused across the chunks
            # of a unit (LDWEIGHTS costs ~50ns of PE stream time each).
            # tanh on ACT, x v_o and the ot-accumulation on DVE.
            b, s0, sz = unit
            acc = acc_p.tile([P, sz], BF16, name="acc")
            for ot in range(n_ot):
                eng = en_p.tile([P, sz], BF16, name="eng")
                pe = pe_p.tile([P, sz], F32, name="pe")
                for hc in range(n_hc):
                    for c0 in range(0, sz, 512):
                        nc.tensor.matmul(
                            pe[:, c0:c0 + 512],
                            wt_bf[:, n_hc + hc, ot * P:(ot + 1) * P],
                            eT[:, hc, c0:c0 + 512],
                            start=(hc == 0), stop=(hc == n_hc - 1))
                nc.scalar.activation(
                    eng[:], pe[:], AF.Tanh, bias=hb[:, ot, b:b + 1])
                if ot == 0:
                    nc.vector.tensor_scalar_mul(acc[:], eng[:], vt_f[:, 0:1])
                else:
                    tmp = tmp_p.tile([P, sz], BF16, name="tmp")
                    nc.vector.tensor_scalar_mul(
                        tmp[:], eng[:], vt_f[:, ot:ot + 1])
                    nc.vector.tensor_tensor(acc[:], acc[:], tmp[:], ADD)
            return acc

        def phase3_fin(unit, acc):
            # partition-reduce via ones-matmul (emitted one unit late so
            # the PE never waits on the DVE acc), then DVE copies the
            # logits PSUM->SBUF.  This copy is the only op that waits on
            # the late ones-matmul, and it sits LAST in the DVE FIFO for
            # this iteration, so nothing upstream ever blocks behind it.
            b, s0, sz = unit
            for c0 in range(0, sz, 512):
                pa = pa_p.tile([P, 512], F32, name="pa")
                nc.tensor.matmul(
                    pa[0:1, :], ones_bf[:, 0:1], acc[:, c0:c0 + 512],
                    start=True, stop=True)
                nc.vector.tensor_copy(
                    out=att_rows[b][0:1, s0 + c0:s0 + c0 + 512],
                    in_=pa[0:1, :])

        def softmax_row(b):
            # Runs 2+ units after row b's logits landed in SBUF, so every
            # input is long ready when each queue reaches these ops.
            # |att| < ~6, so exp() is safe in fp32 without the row max.
            e_row = row_p.tile([1, s], F32, name="erow")
            ssum = const.tile([1, 1], F32, name=f"ssum{b}")
            nc.scalar.activation(
                e_row[:], att_rows[b][:], AF.Exp, accum_out=ssum[:])
            rinv = const.tile([1, 1], F32, name=f"rinv{b}")
            nc.vector.reciprocal(rinv[:], ssum[:])
            nc.vector.tensor_scalar_mul(e_row[:], e_row[:], rinv[:])
            nc.gpsimd.dma_start(out[b:b + 1, :], e_row[:])

        # ---- software pipeline, 3-unit load lookahead:
        #   iter i: transposes(u_{i+1}) | matmuls(u_i) | loads+casts(u_{i+3})
        #           | fin(u_{i-1}) | softmax(row done at u_{i-2})
        # Transposes are emitted BEFORE the matmuls/activations of u_i so
        # they sit at the head of the scalar FIFO and dispatch while the
        # PE is still streaming u_i; their only waits (cast done, encT
        # buffer freed by u_{i-1}'s last matmul) fire earlier than the
        # ACTIVATE(u_i) deps queued behind them.
        LOOK = 2
        bts_q = {}
        bts_q[0] = phase1(units[0])
        eT_cur = phase2(units[0], bts_q.pop(0))
        emit_w(0)
        emit_w(1)
        hb = emit_hproj()
        for k in range(1, min(LOOK, len(units))):
            bts_q[k] = phase1(units[k])

        fin = None
        sm_row = None
        for i, u in enumerate(units):
            eT_next = None
            if i + 1 < len(units):
                eT_next = phase2(units[i + 1], bts_q.pop(i + 1))
            acc = phase3_mm(u, eT_cur, hb)
            eT_cur = eT_next
            if i + LOOK < len(units):
                bts_q[i + LOOK] = phase1(units[i + LOOK])
            if sm_row is not None:
                softmax_row(sm_row)
                sm_row = None
            if fin is not None:
                phase3_fin(*fin)
                fb, fs0, fsz = fin[0]
                if fs0 + fsz == s:
                    sm_row = fb
            fin = (u, acc)
            if i == 0:
                keepalive(startup_keep)
            else:
                keepalive(keepalive_mm)
        phase3_fin(*fin)
        softmax_row(b_loc - 1)

    nc.compile()
    return nc


def make_in_maps(hidden, encoder_outputs, W_attn, b_attn, v, n_cores=8):
    hidden = np.asarray(hidden, dtype=np.float32)
    encoder_outputs = np.asarray(encoder_outputs, dtype=np.float32)
    W_attn = np.asarray(W_attn, dtype=np.float32)
    b_attn = np.asarray(b_attn, dtype=np.float32)
    v = np.asarray(v, dtype=np.float32)

    b = encoder_outputs.shape[0]
    b_loc = b // n_cores
    wt = np.ascontiguousarray(W_attn.T.astype(ml_dtypes.bfloat16))
    ones = np.ones((P, 1), dtype=ml_dtypes.bfloat16)
    in_maps = []
    for i in range(n_cores):
        bsl = slice(b_loc * i, b_loc * (i + 1))
        in_maps.append({
            "wt": wt,
            "hiddenT": np.ascontiguousarray(
                hidden[0, bsl].T.astype(ml_dtypes.bfloat16)),
            "b_attn": b_attn,
            "v": v,
            "ones": ones,
            "zeros": np.zeros((P, 512), dtype=ml_dtypes.bfloat16),
            "enc": np.ascontiguousarray(encoder_outputs[bsl]),
        })
    return in_maps


_NC_CACHE = {}


def _get_nc():
    if "nc" not in _NC_CACHE:
        _NC_CACHE["nc"] = build_nc(b_loc=4, s=2048, h=1024, n_cores=8)
    return _NC_CACHE["nc"]


def kernel(hidden, encoder_outputs, W_attn, b_attn, v):
    from concourse.bass_utils import run_bass_kernel_spmd

    nc = _get_nc()
    in_maps = make_in_maps(hidden, encoder_outputs, W_attn, b_attn, v,
                           n_cores=8)
    res = run_bass_kernel_spmd(nc, in_maps, core_ids=list(range(8)))
    out = np.concatenate([np.asarray(res.results[i]["out"])
                          for i in range(8)], axis=0)
    return out.astype(np.float32)


# revision 19
# speedup vs baseline: 1.0679x; 1.0679x over previous
"""Trainium2 Bass kernel for nn_Attention_85074712199827.

Computes, for hidden [1,32,1024], encoder_outputs [32,2048,1024],
W_attn [1024,2048], b_attn [1024], v [1024]:

    h_proj  = hidden[0] @ W_attn[:, :1024].T
    e_proj  = encoder_outputs @ W_attn[:, 1024:].T
    energy  = tanh(e_proj + h_proj[:, None, :] + b_attn)
    att     = energy @ v
    out     = softmax(att, axis=1)          # [32, 2048] float32

Distribution: data-parallel over the batch across 8 NeuronCores (4
batch rows per core); the tiny parameters are replicated (pre-laid-out
and pre-cast to bf16 on the host). Each core runs an independent
Bass/Tile program; results are concatenated on the host.

Per-core schedule: enc streams HBM->SBUF fp32 on the sync HWDGE queue
(loads only -- nothing else shares that ring), is cast to bf16 (DVE),
and xbar-transposed to [h, s] layout on the scalar HWDGE queue.  The
e_proj matmuls run in 1024-wide units with the stationary weight block
reused across the two 512-wide PSUM halves, halving LDWEIGHTS
pressure (each LDWEIGHTS steals ~50ns from the concurrent matmul).
The v-dot is NOT done with 1-row PE matmuls: the tanh output is scaled
by v on the scalar engine, the 8 o-chunks are summed on DVE, and a
single ones-vector matmul reduces the 128 partitions.  Softmax runs
per batch row as soon as that row's attention logits are complete,
overlapped with the next row's matmuls.

Self-contained: only environment packages (concourse, numpy, ml_dtypes)
are imported; all shapes/sharding are hardcoded for this problem.
"""

from contextlib import ExitStack

import ml_dtypes
import numpy as np

import concourse.bass as bass
import concourse.tile as tile
from concourse import bacc, mybir

F32 = mybir.dt.float32
BF16 = mybir.dt.bfloat16
AF = mybir.ActivationFunctionType
ADD = mybir.AluOpType.add
P = 128


def build_nc(b_loc=4, s=2048, h=1024, n_cores=8, sb=1024,
             warmup_mm=30, startup_keep=8, keepalive_mm=0,
             first_sb=512, ld_chunk=512):
    n_hc = h // P           # contraction chunks
    n_ot = h // P           # output (o) tiles

    nc = bacc.Bacc("TRN2", target_bir_lowering=False, debug=False,
                   num_devices=n_cores)

    wt = nc.dram_tensor("wt", [2 * h, h], BF16, kind="ExternalInput").ap()
    hiddenT = nc.dram_tensor("hiddenT", [h, b_loc], BF16, kind="ExternalInput").ap()
    b_attn = nc.dram_tensor("b_attn", [h], F32, kind="ExternalInput").ap()
    v = nc.dram_tensor("v", [h], F32, kind="ExternalInput").ap()
    ones = nc.dram_tensor("ones", [P, 1], BF16, kind="ExternalInput").ap()
    zeros = nc.dram_tensor("zeros", [P, 512], BF16, kind="ExternalInput").ap()
    enc = nc.dram_tensor("enc", [b_loc, s, h], F32, kind="ExternalInput").ap()
    out = nc.dram_tensor("out", [b_loc, s], F32, kind="ExternalOutput").ap()

    with tile.TileContext(nc) as tc, ExitStack() as ctx:
        const = ctx.enter_context(tc.tile_pool(name="const", bufs=1))
        psmall = ctx.enter_context(tc.tile_pool(name="psmall", bufs=1, space="PSUM"))

        # ---- PE warmup: dependency-free matmuls to lift the HAM clock
        # gate to 8/8 while the first enc block is still in flight ----
        wz = const.tile([P, 512], BF16)
        nc.scalar.dma_start(wz[:], zeros)
        for i in range(warmup_mm):
            pw = psmall.tile([P, 512], F32, name="pw", tag="ps")
            nc.tensor.matmul(pw[:], wz[:, :P], wz[:], start=True, stop=True)

        def keepalive(n):
            for _ in range(n):
                pw = psmall.tile([P, 512], F32, name="pw", tag="ps")
                nc.tensor.matmul(pw[:], wz[:, :P], wz[:], start=True, stop=True)

        # ---- small constants (scalar queue; off the critical path) ----
        hT_bf = const.tile([P, n_hc, b_loc], BF16)
        nc.scalar.dma_start(hT_bf[:], hiddenT.rearrange("(hc p) b -> p hc b", p=P))

        baT = const.tile([P, n_ot], F32)
        nc.scalar.dma_start(baT[:], b_attn.rearrange("(oc p) -> p oc", p=P))

        vt_f = const.tile([P, n_ot], F32)
        nc.scalar.dma_start(vt_f[:], v.rearrange("(oc p) -> p oc", p=P))

        ones_bf = const.tile([P, 1], BF16)
        nc.scalar.dma_start(ones_bf[:], ones)

        # ---- weights: W_attn.T arrives [2h, h] bf16; We half first so
        # e_proj unblocks while Wh still streams ----
        wt_bf = const.tile([P, 2 * n_hc, h], BF16)
        wt_r = wt.rearrange("(jc p) o -> p jc o", p=P)
        q = n_hc // 2

        def emit_w(c):
            nc.scalar.dma_start(
                wt_bf[:, c * q:(c + 1) * q, :],
                wt_r[:, c * q:(c + 1) * q, :])

        emit_w(2)
        emit_w(3)

        def emit_hproj(ph_pool):
            # all 64 tiny matmuls into TWO psum tiles, alternating per ot
            # (consecutive same-bank matmuls pay the ~50ns hazard; and a
            # 1-buf pool with per-ot DVE round-trips cost ~15us of PE
            # dribble at startup), then two DVE adds fold in b_attn.
            hb = const.tile([P, n_ot, b_loc], F32, name="hb")
            nh = n_ot // 2
            phs = [ph_pool.tile([P, nh, b_loc], F32, name="ph", tag="pa")
                   for _ in range(2)]
            for op in range(nh):
                for hc in range(n_hc):
                    for par in range(2):
                        ot = 2 * op + par
                        nc.tensor.matmul(
                            phs[par][:, op, :],
                            wt_bf[:, hc, ot * P:(ot + 1) * P],
                            hT_bf[:, hc, :],
                            start=(hc == 0), stop=(hc == n_hc - 1))
            for par in range(2):
                nc.vector.tensor_tensor(
                    hb[:, par::2, :], phs[par][:],
                    baT[:, par::2, None].to_broadcast((P, nh, b_loc)),
                    mybir.AluOpType.add)
            return hb

        # ---- main pipeline pools ----
        inp = ctx.enter_context(tc.tile_pool(name="inp", bufs=2))
        bfp = ctx.enter_context(tc.tile_pool(name="bfp", bufs=4))
        encT_p = ctx.enter_context(tc.tile_pool(name="encT", bufs=2))
        en_p = ctx.enter_context(tc.tile_pool(name="energy", bufs=3))
        tmp_p = ctx.enter_context(tc.tile_pool(name="vtmp", bufs=3))
        acc_p = ctx.enter_context(tc.tile_pool(name="acc", bufs=3))
        row_p = ctx.enter_context(tc.tile_pool(name="rowbuf", bufs=1))
        pe_p = ctx.enter_context(tc.tile_pool(name="psum_e", bufs=2, space="PSUM"))
        pa_p = ctx.enter_context(tc.tile_pool(name="psum_att", bufs=2, space="PSUM"))

        # per-row logits buffers (DVE/ACT accesses must start at partition 0)
        att_rows = [const.tile([1, s], F32, name=f"attrow{b}")
                    for b in range(b_loc)]

        # units: (b, s0, size) — b-major so softmax(b) pipelines.
        # The first units of b=0 are small for a fast pipeline rampup.
        units = []
        for b in range(b_loc):
            if b == 0 and first_sb < sb:
                for s0 in range(0, sb, first_sb):
                    units.append((b, s0, first_sb))
                for s0 in range(sb, s, sb):
                    units.append((b, s0, sb))
            else:
                for s0 in range(0, s, sb):
                    units.append((b, s0, sb))

        def phase1(unit):
            # HBM -> SBUF fp32 loads on the sync HWDGE queue (SWDGE issue
            # on gpsimd measured 10-40us/load), then DVE casts to bf16.
            # The very first unit loads in 128-row chunks so its first
            # transpose can start ~3us in instead of ~8us.
            b, s0, sz = unit
            chunk = P if (b == 0 and s0 == 0) else ld_chunk
            its = []
            for c0 in range(0, sz, chunk):
                csz = min(chunk, sz - c0)
                it = inp.tile([P, csz // P, h], F32, name="it")
                nc.sync.dma_start(
                    it[:], enc[b, s0 + c0:s0 + c0 + csz, :].rearrange(
                        "(si p) h -> p si h", p=P))
                its.append(it)
            bts = []
            for it in its:
                bt = bfp.tile([P, it.shape[1], h], BF16, name="bt")
                nc.vector.tensor_copy(out=bt[:], in_=it[:])
                bts.append(bt)
            return bts

        def phase2(unit, bts):
            # SBUF xbar transpose [s,h] bf16 -> [h,s] on the scalar HWDGE
            # queue so the sync ring only ever carries the enc loads.
            # Returns (eT, thunks): the transposes are emitted lazily by
            # phase3_mm between its ACTIVATEs -- emitting them as one
            # block head-of-line-blocked the ACTIVATEs behind them for
            # ~5us per unit (PSUM backpressure then stalled the PE).
            b, s0, sz = unit
            eT = encT_p.tile([P, n_hc, sz], BF16, name="eT")
            thunks = []
            col = 0
            for bt in bts:
                for si in range(bt.shape[1]):
                    def t(bt=bt, si=si, col=col):
                        nc.scalar.dma_start_transpose(
                            eT[:, :, col:col + P], bt[:, si, :])
                    thunks.append(t)
                    col += P
            return eT, thunks

        def phase3_mm(unit, eT, hb, next_thunks=()):
            # PSUM matmul output must fit one 2KB bank -> 512-wide chunks.
            # Consecutive matmuls MUST alternate PSUM banks: back-to-back
            # accumulation into the same bank costs ~50ns per matmul
            # (264ns vs 213ns measured), hence `for hc: for c0`.
            # tanh on ACT, x v_o and the ot-accumulation on DVE.  The
            # next unit's transposes are sprinkled between the first
            # ACTIVATEs (2 each) so they dispatch early but never
            # head-of-line-block an ACTIVATE whose PSUM bank the PE is
            # waiting to reuse.
            b, s0, sz = unit
            acc = acc_p.tile([P, sz], BF16, name="acc")
            tq = list(next_thunks)
            for ot in range(n_ot):
                eng = en_p.tile([P, sz], BF16, name="eng")
                pe = pe_p.tile([P, sz], F32, name="pe")
                for hc in range(n_hc):
                    for c0 in range(0, sz, 512):
                        nc.tensor.matmul(
                            pe[:, c0:c0 + 512],
                            wt_bf[:, n_hc + hc, ot * P:(ot + 1) * P],
                            eT[:, hc, c0:c0 + 512],
                            start=(hc == 0), stop=(hc == n_hc - 1))
                nc.scalar.activation(
                    eng[:], pe[:], AF.Tanh, bias=hb[:, ot, b:b + 1])
                for _ in range(2):
                    if tq:
                        tq.pop(0)()
                if ot == 0:
                    nc.vector.tensor_scalar_mul(acc[:], eng[:], vt_f[:, 0:1])
                else:
                    tmp = tmp_p.tile([P, sz], BF16, name="tmp")
                    nc.vector.tensor_scalar_mul(
                        tmp[:], eng[:], vt_f[:, ot:ot + 1])
                    nc.vector.tensor_tensor(acc[:], acc[:], tmp[:], ADD)
            for t in tq:
                t()
            return acc

        def phase3_fin(unit, acc):
            # partition-reduce via ones-matmul (emitted one unit late so
            # the PE never waits on the DVE acc), then DVE copies the
            # logits PSUM->SBUF.  This copy is the only op that waits on
            # the late ones-matmul, and it sits LAST in the DVE FIFO for
            # this iteration, so nothing upstream ever blocks behind it.
            b, s0, sz = unit
            for c0 in range(0, sz, 512):
                pa = pa_p.tile([P, 512], F32, name="pa", tag="pa")
                nc.tensor.matmul(
                    pa[0:1, :], ones_bf[:, 0:1], acc[:, c0:c0 + 512],
                    start=True, stop=True)
                nc.vector.tensor_copy(
                    out=att_rows[b][0:1, s0 + c0:s0 + c0 + 512],
                    in_=pa[0:1, :])

        def softmax_row(b):
            # Runs 2+ units after row b's logits landed in SBUF, so every
            # input is long ready when each queue reaches these ops.
            # |att| < ~6, so exp() is safe in fp32 without the row max.
            e_row = row_p.tile([1, s], F32, name="erow")
            ssum = const.tile([1, 1], F32, name=f"ssum{b}")
            nc.scalar.activation(
                e_row[:], att_rows[b][:], AF.Exp, accum_out=ssum[:])
            rinv = const.tile([1, 1], F32, name=f"rinv{b}")
            nc.vector.reciprocal(rinv[:], ssum[:])
            nc.vector.tensor_scalar_mul(e_row[:], e_row[:], rinv[:])
            nc.gpsimd.dma_start(out[b:b + 1, :], e_row[:])

        # ---- software pipeline, 2-unit load lookahead:
        #   iter i: matmuls(u_i) + interleaved transposes(u_{i+1})
        #           | loads+casts(u_{i+2}) | fin(u_{i-1}) | softmax
        # All four weight quarters stream before the first transposes so
        # nothing on the scalar ring delays them, and h_proj's matmuls
        # only wait on the Wh quarters, not on a DVE round-trip.
        LOOK = 2
        bts_q = {}
        bts_q[0] = phase1(units[0])
        emit_w(0)
        emit_w(1)
        eT_cur, t0_thunks = phase2(units[0], bts_q.pop(0))
        for t in t0_thunks:
            t()
        hb = emit_hproj(pa_p)
        for k in range(1, min(LOOK, len(units))):
            bts_q[k] = phase1(units[k])

        fin = None
        sm_row = None
        for i, u in enumerate(units):
            eT_next = None
            thunks = ()
            if i + 1 < len(units):
                eT_next, thunks = phase2(units[i + 1], bts_q.pop(i + 1))
            acc = phase3_mm(u, eT_cur, hb, thunks)
            eT_cur = eT_next
            if i + LOOK < len(units):
                bts_q[i + LOOK] = phase1(units[i + LOOK])
            if sm_row is not None:
                softmax_row(sm_row)
                sm_row = None
            if fin is not None:
                phase3_fin(*fin)
                fb, fs0, fsz = fin[0]
                if fs0 + fsz == s:
                    sm_row = fb
            fin = (u, acc)
            if i == 0:
                keepalive(startup_keep)
            else:
                keepalive(keepalive_mm)
        phase3_fin(*fin)
        softmax_row(b_loc - 1)

    nc.compile()
    return nc


def make_in_maps(hidden, encoder_outputs, W_attn, b_attn, v, n_cores=8):
    hidden = np.asarray(hidden, dtype=np.float32)
    encoder_outputs = np.asarray(encoder_outputs, dtype=np.float32)
    W_attn = np.asarray(W_attn, dtype=np.float32)
    b_attn = np.asarray(b_attn, dtype=np.float32)
    v = np.asarray(v, dtype=np.float32)

    b = encoder_outputs.shape[0]
    b_loc = b // n_cores
    wt = np.ascontiguousarray(W_attn.T.astype(ml_dtypes.bfloat16))
    ones = np.ones((P, 1), dtype=ml_dtypes.bfloat16)
    in_maps = []
    for i in range(n_cores):
        bsl = slice(b_loc * i, b_loc * (i + 1))
        in_maps.append({
            "wt": wt,
            "hiddenT": np.ascontiguousarray(
                hidden[0, bsl].T.astype(ml_dtypes.bfloat16)),
            "b_attn": b_attn,
            "v": v,
            "ones": ones,
            "zeros": np.zeros((P, 512), dtype=ml_dtypes.bfloat16),
            "enc": np.ascontiguousarray(encoder_outputs[bsl]),
        })
    return in_maps


_NC_CACHE = {}


def _get_nc():
    if "nc" not in _NC_CACHE:
        _NC_CACHE["nc"] = build_nc(b_loc=4, s=2048, h=1024, n_cores=8)
    return _NC_CACHE["nc"]


def kernel(hidden, encoder_outputs, W_attn, b_attn, v):
    from concourse.bass_utils import run_bass_kernel_spmd

    nc = _get_nc()
    in_maps = make_in_maps(hidden, encoder_outputs, W_attn, b_attn, v,
                           n_cores=8)
    res = run_bass_kernel_spmd(nc, in_maps, core_ids=list(range(8)))
    out = np.concatenate([np.asarray(res.results[i]["out"])
                          for i in range(8)], axis=0)
    return out.astype(np.float32)
